# revision 12
# baseline (speedup 1.0000x reference)
"""KeOps-style multi-head attention (unnormalized-exp softmax) on 8 trn2 cores.

Sharding: core c handles batch bi = c//2 and query rows u*1024..(u+1)*1024
(u = c%2), ALL 8 heads. Output is a pure concat over cores (no reduction).

All matmul operands are bf16 (PE runs 1 cycle/row vs 4 for fp32); every
accumulation (PSUM), the softmax denominator path and the final output stay
fp32, keeping the end-to-end rel error ~5e-3.

Per-core pipeline (one uniform SPMD program):
  A) DMA x as bf16 (full batch rows for K/V, this core's rows for Q) +
     weights; transpose x on PE (128x128 identity-matmul transposes,
     bf16 1 cyc/row), 8 transposes batched per DVE evacuation.
  B) QKV projections (1024-wide bf16 moving operands) producing q^T/k^T in
     "stacked head" layout [32*h_local + d, n] (heads packed 4-per-tensor so
     the K=32 scores matmuls can be row-tiled 4x on the PE array), and v in
     normal layout with a ones-column appended (gives the softmax denominator
     for free from the numerator matmul). v evacuation runs on the Scalar
     engine (idle during the prologue) to keep DVE off the critical path.
  C) Attention: scores^T chunks [nk=128, nq=1024] via 4x row-tiled K=32
     bf16 matmuls (each into its own PSUM bank) -> exp on ACT
     (psum->sbuf bf16, [128,1024] per instr, the bottleneck engine) ->
     numer^T accumulation with e^T as a 1024-wide moving operand against a
     merged two-head stationary [128, 66] (half the instruction count; the
     off-diagonal quadrants are discarded at evacuation).
  D) Normalize by 1/(denom+eps): numer/denom evacuate in fp32, the DVE
     normalize multiply emits bf16 PT directly; project with Wout + bias.
"""

import numpy as np
import ml_dtypes
from contextlib import ExitStack

import concourse.bass as bass
import concourse.mybir as mybir
import concourse.tile as tile
from concourse import bacc
from concourse.bass_utils import run_bass_kernel_spmd

DIM = 256
NUM_HEADS = 8
HEAD_DIM = 32
B = 4
N = 2048
NQ = 1024          # query rows per core
NCORES = 8
FP = mybir.dt.float32
BF = mybir.dt.bfloat16
EXP = mybir.ActivationFunctionType.Exp
CPY = mybir.ActivationFunctionType.Copy
LN = mybir.ActivationFunctionType.Ln
I16 = mybir.dt.int16
MULT = mybir.AluOpType.mult
ADD = mybir.AluOpType.add

NT_KV = N // 128   # 16 n-tiles of kv rows
NT_Q = NQ // 128   # 8 n-tiles of q rows
NGQ = NQ // 512    # 2 groups of 512 query cols in q^T
NJ = N // 128      # 16 key chunks of 128
NDUMMY = 2         # dummy LDWEIGHTS per j to hold the HAM clock at 8/8
DVE_PICK = {1, 4, 6}   # (2j+p)%8 residues whose exp runs on DVE (Schraudolph)
SCHRAU_A = float(2 ** 7 / np.log(2.0))
SCHRAU_B = float(127 * 2 ** 7 - 8.0)


def build_program():
    nc = bacc.Bacc()

    xq = nc.declare_dram_parameter("xq", [NQ, DIM], BF, isOutput=False)
    xkv = nc.declare_dram_parameter("xkv", [N, DIM], BF, isOutput=False)
    wq = nc.declare_dram_parameter("wq", [DIM, DIM], BF, isOutput=False)
    wk = nc.declare_dram_parameter("wk", [DIM, DIM], BF, isOutput=False)
    wv = nc.declare_dram_parameter("wv", [DIM, DIM], BF, isOutput=False)
    wout = nc.declare_dram_parameter("wout", [DIM, DIM], BF, isOutput=False)
    bout = nc.declare_dram_parameter("bout", [DIM], FP, isOutput=False)
    ident_in = nc.declare_dram_parameter("ident", [128, 128], BF, isOutput=False)
    out = nc.declare_dram_parameter("out", [NQ, DIM], FP, isOutput=True)

    with tile.TileContext(nc) as tc, ExitStack() as ctx:
        consts = ctx.enter_context(tc.tile_pool(name="consts", bufs=1))
        persist = ctx.enter_context(tc.tile_pool(name="persist", bufs=1))

        ident = consts.tile([128, 128], BF)
        nc.sync.dma_start(out=ident, in_=ident_in[:, :])
        bias_b = consts.tile([128, DIM], FP)
        nc.sync.dma_start(out=bias_b, in_=bout[:].unsqueeze(0).to_broadcast([128, DIM]))
        # Warm the ACT table RAM with the ln+exp set during the prologue
        # (Ln only lives in natural_log_exp_and_others, which also has Exp,
        # so the whole kernel runs on a single ~2.7us table load).
        tbl_warm = consts.tile([1, DIM], FP)
        nc.scalar.activation(tbl_warm, bias_b[0:1, :], LN)

        # ---- weights ----
        # wq_sb/wk_sb/wv_sb: [128(c_local), ck, m]; lhsT slices are [128, 128]
        wq_sb = consts.tile([128, 2, DIM], BF)
        wk_sb = consts.tile([128, 2, DIM], BF)
        wv_sb = consts.tile([128, 2, DIM], BF)
        wout_sb = consts.tile([128, 2, DIM], BF)
        for ck in range(2):
            nc.sync.dma_start(out=wq_sb[:, ck, :], in_=wq[128 * ck:128 * (ck + 1), :])
            nc.sync.dma_start(out=wk_sb[:, ck, :], in_=wk[128 * ck:128 * (ck + 1), :])
            nc.sync.dma_start(out=wv_sb[:, ck, :], in_=wv[128 * ck:128 * (ck + 1), :])
            nc.sync.dma_start(out=wout_sb[:, ck, :], in_=wout[128 * ck:128 * (ck + 1), :])

        # ---- x loads (per 128-row tile so transposes can start early) ----
        xkv_sb = persist.tile([128, NT_KV, DIM], BF)
        for t in range(NT_KV):
            nc.sync.dma_start(out=xkv_sb[:, t, :], in_=xkv[128 * t:128 * (t + 1), :])
        xq_sb = persist.tile([128, NT_Q, DIM], BF)
        for t in range(NT_Q):
            nc.sync.dma_start(out=xq_sb[:, t, :], in_=xq[128 * t:128 * (t + 1), :])

        # ---- transposes: xkvT [128(c_local), ck, n], xqT [128, ck, nq] ----
        # 8 transposes share one [128, 1024] bf16 psum tile (sequential
        # writes to one bank), evacuated by a single DVE copy.
        xkvT = persist.tile([128, 2, N], BF)
        xqT = persist.tile([128, 2, NQ], BF)
        with tc.tile_pool(name="tps", bufs=3, space="PSUM") as tps:
            for ck in range(2):
                for b8 in range(NT_KV // 8):
                    ps = tps.tile([128, 1024], BF, tag="tp")
                    for k in range(8):
                        t = 8 * b8 + k
                        nc.tensor.transpose(
                            ps[:, 128 * k:128 * (k + 1)],
                            xkv_sb[:, t, 128 * ck:128 * (ck + 1)], ident)
                    nc.vector.tensor_copy(
                        xkvT[:, ck, 1024 * b8:1024 * (b8 + 1)], ps)
                ps = tps.tile([128, 1024], BF, tag="tp")
                for k in range(NT_Q):
                    nc.tensor.transpose(
                        ps[:, 128 * k:128 * (k + 1)],
                        xq_sb[:, k, 128 * ck:128 * (ck + 1)], ident)
                nc.vector.tensor_copy(xqT[:, ck, :], ps)

        # ---- QKV projections ----
        # qT/kT stacked-head layout: tensor i in {0,1} holds heads 4i..4i+3:
        # row 32*hloc + d  <->  head 4i+hloc, dim d.
        qT = [persist.tile([128, NQ], BF, tag=f"qT{i}", name=f"qT{i}") for i in range(2)]
        kT = [persist.tile([128, N], BF, tag=f"kT{i}", name=f"kT{i}") for i in range(2)]
        # v normal layout + ones column: [128(n), t, h, 33]
        v_sb = persist.tile([128, NT_KV, NUM_HEADS, HEAD_DIM + 1], BF)
        nc.vector.memset(v_sb[:, :, :, HEAD_DIM:], 1.0)

        with (
            tc.tile_pool(name="qkvp", bufs=3, space="PSUM") as qkvp,
            tc.tile_pool(name="vp", bufs=2, space="PSUM") as vp,
        ):
            for i in range(2):
                for g in range(NGQ):
                    ps = qkvp.tile([128, 512], FP, tag="proj")
                    for ck in range(2):
                        nc.tensor.matmul(
                            ps, lhsT=wq_sb[:, ck, 128 * i:128 * (i + 1)],
                            rhs=xqT[:, ck, 512 * g:512 * (g + 1)],
                            start=(ck == 0), stop=(ck == 1))
                    nc.vector.tensor_copy(qT[i][:, 512 * g:512 * (g + 1)], ps)
                for g in range(N // 512):
                    ps = qkvp.tile([128, 512], FP, tag="proj")
                    for ck in range(2):
                        nc.tensor.matmul(
                            ps, lhsT=wk_sb[:, ck, 128 * i:128 * (i + 1)],
                            rhs=xkvT[:, ck, 512 * g:512 * (g + 1)],
                            start=(ck == 0), stop=(ck == 1))
                    nc.vector.tensor_copy(kT[i][:, 512 * g:512 * (g + 1)], ps)
            for t in range(NT_KV):
                ps = vp.tile([128, DIM], FP, tag="vproj")
                for ck in range(2):
                    nc.tensor.matmul(
                        ps, lhsT=xkvT[:, ck, 128 * t:128 * (t + 1)],
                        rhs=wv_sb[:, ck, :],
                        start=(ck == 0), stop=(ck == 1))
                # strided copy into the 33-wide per-head slots, on the Scalar
                # engine (idle during the prologue; DVE handles q/k evacs)
                nc.scalar.activation(v_sb[:, t, :, 0:HEAD_DIM], ps, CPY)

        # ---- attention ----
        # PT: normalized pre-projection, transposed: tensor i rows = wout rows
        # 128i..128i+128 (head 4i+hloc dim d at partition 32*hloc+d).
        PTf = [persist.tile([128, NQ], FP, tag=f"PTf{i}", name=f"PTf{i}") for i in range(2)]
        PTb = [persist.tile([128, NQ], BF, tag=f"PTb{i}", name=f"PTb{i}") for i in range(2)]
        # per-block denominators (engine reads must start at a 32-aligned
        # partition, so each (g,hh) block gets its own [4,512] tile)
        denoms = [persist.tile([4, 512], FP, tag=f"dn{b}", name=f"dn{b}")
                  for b in range(4)]

        with (
            tc.tile_pool(name="spsum", bufs=2, space="PSUM") as spsum,
            tc.tile_pool(name="npsum", bufs=1, space="PSUM") as npsum,
            tc.tile_pool(name="esb", bufs=4) as esb,
            tc.tile_pool(name="evac", bufs=4) as evac,
            tc.tile_pool(name="rbp", bufs=2) as rbp,
            tc.tile_pool(name="rcpp", bufs=2) as rcpp,
            tc.tile_pool(name="dscratch", bufs=2, space="DRAM") as dsc,
        ):
            def normalize_block(g, hh):
                # Per-block softmax normalize, fully overlapped with the next
                # block's attention. Reciprocal via ln -> exp(-x) on ACT (the
                # natural_log_exp_and_others table set holds both functions,
                # and the DVE reciprocal instruction costs ~4us). The
                # normalize multiply runs on the otherwise-idle GpSimd.
                dn = denoms[2 * g + hh]
                rcp = rcpp.tile([4, 512], FP, tag="rcp")
                nc.scalar.activation(rcp, dn, LN)
                nc.scalar.activation(rcp, rcp, EXP, scale=-1.0)
                recip_dram = dsc.tile([4, 512], FP, tag="rd")
                nc.sync.dma_start(out=recip_dram[:, :], in_=rcp)
                rb = rbp.tile([128, 512], FP, tag="rb")
                for hloc in range(4):
                    nc.sync.dma_start(
                        out=rb[32 * hloc:32 * hloc + 32, :],
                        in_=recip_dram[hloc:hloc + 1, :].to_broadcast([32, 512]))
                nc.gpsimd.tensor_mul(PTb[hh][:, 512 * g:512 * (g + 1)],
                                     PTf[hh][:, 512 * g:512 * (g + 1)], rb)

            pending_norm = None
            for g in range(NGQ):
                for hh in range(2):
                    nps = [npsum.tile([HEAD_DIM + 1, 512], FP,
                                      tag=f"np{x}", name=f"np{x}") for x in range(4)]

                    def numers(j, es):
                        for p in range(2):
                            for uu in range(2):
                                hloc = 2 * p + uu
                                h = 4 * hh + hloc
                                nc.tensor.matmul(
                                    nps[hloc],
                                    lhsT=v_sb[:, j, h, :],
                                    rhs=es[p][:, 512 * uu:512 * (uu + 1)],
                                    start=(j == 0), stop=(j == NJ - 1))

                    # Software-pipelined by one j: scores(j) [4x row-tiled,
                    # all concurrent] -> exp(j) -> numer(j-1), so the PE
                    # fills the exp latency with independent score work.
                    # exp runs on ACT except for DVE_PICK residues, which use
                    # a one-instruction Schraudolph exp on DVE:
                    #   bf16(exp(x)) ~= bitcast_bf16(int16(x*128/ln2 + Bc))
                    # Dummy LDWEIGHTS after each numer batch keep the PE
                    # continuously busy so the HAM clock gate stays at 8/8
                    # (any recurring idle gap drops the PE to 1.2 GHz).
                    prev = None
                    for j in range(NJ):
                        sps = []
                        for p in range(2):
                            sp = spsum.tile([128, 1024], FP, tag="sp")
                            for uu in range(2):
                                hloc = 2 * p + uu
                                r = 32 * hloc
                                nc.tensor.matmul(
                                    sp[:, 512 * uu:512 * (uu + 1)],
                                    lhsT=kT[hh][r:r + 32, 128 * j:128 * (j + 1)],
                                    rhs=qT[hh][r:r + 32, 512 * g:512 * (g + 1)],
                                    start=True, stop=True,
                                    tile_position=(r, 0))
                            sps.append(sp)
                        es = []
                        for p in range(2):
                            e = esb.tile([128, 1024], BF, tag="e")
                            if (2 * j + p) % 8 in DVE_PICK:
                                nc.vector.tensor_scalar(
                                    out=e.bitcast(I16), in0=sps[p],
                                    scalar1=SCHRAU_A, scalar2=SCHRAU_B,
                                    op0=MULT, op1=ADD)
                            else:
                                nc.scalar.activation(e, sps[p], EXP)
                            es.append(e)
                        if prev is not None:
                            numers(*prev)
                        if j == 0 and pending_norm is not None:
                            normalize_block(*pending_norm)
                        for _ in range(NDUMMY):
                            nc.tensor.ldweights(ident)
                        prev = (j, es)
                    numers(*prev)
                    for hloc in range(4):
                        tmp = evac.tile([HEAD_DIM + 1, 512], FP, tag="ev")
                        nc.vector.tensor_copy(tmp, nps[hloc])
                        nc.sync.dma_start(
                            out=PTf[hh][32 * hloc:32 * hloc + 32,
                                        512 * g:512 * (g + 1)],
                            in_=tmp[0:HEAD_DIM, :])
                        nc.sync.dma_start(
                            out=denoms[2 * g + hh][hloc:hloc + 1, :],
                            in_=tmp[HEAD_DIM:HEAD_DIM + 1, :])
                    pending_norm = (g, hh)
            normalize_block(*pending_norm)

        # ---- output projection ----
        with (
            tc.tile_pool(name="opsum", bufs=4, space="PSUM") as opsum,
            tc.tile_pool(name="osb", bufs=4) as osb,
        ):
            for t in range(NT_Q):
                ps = opsum.tile([128, DIM], FP, tag="o")
                for i in range(2):
                    nc.tensor.matmul(
                        ps, lhsT=PTb[i][:, 128 * t:128 * (t + 1)],
                        rhs=wout_sb[:, i, :],
                        start=(i == 0), stop=(i == 1))
                ob = osb.tile([128, DIM], FP, tag="ob")
                nc.vector.tensor_add(ob, ps, bias_b)
                nc.sync.dma_start(out=out[128 * t:128 * (t + 1), :], in_=ob)

    if not nc.is_finalized():
        nc.finalize()
    return nc


_NC_CACHE = None


def _get_program():
    global _NC_CACHE
    if _NC_CACHE is None:
        _NC_CACHE = build_program()
    return _NC_CACHE


def kernel(x, Wqkv, Wout, bout, _trace=False, _trace_kwargs=None):
    bf = ml_dtypes.bfloat16
    x = np.asarray(x, dtype=np.float32)
    Wqkv = np.asarray(Wqkv, dtype=np.float32)
    Wout = np.asarray(Wout, dtype=np.float32)
    bout = np.asarray(bout, dtype=np.float32)

    scale = HEAD_DIM ** -0.5
    wq = np.ascontiguousarray(Wqkv[:, 0:DIM] * scale).astype(bf)
    wk = np.ascontiguousarray(Wqkv[:, DIM:2 * DIM]).astype(bf)
    wv = np.ascontiguousarray(Wqkv[:, 2 * DIM:3 * DIM]).astype(bf)
    wo = np.ascontiguousarray(Wout).astype(bf)
    xb = x.astype(bf)

    in_maps = []
    for c in range(NCORES):
        bi, u = c // 2, c % 2
        in_maps.append({
            "xq": np.ascontiguousarray(xb[bi, u * NQ:(u + 1) * NQ, :]),
            "xkv": np.ascontiguousarray(xb[bi]),
            "wq": wq, "wk": wk, "wv": wv,
            "wout": wo,
            "bout": bout,
            "ident": np.eye(128, dtype=np.float32).astype(bf),
        })

    nc = _get_program()
    kwargs = {}
    if _trace:
        kwargs["trace"] = True
        if _trace_kwargs:
            kwargs.update(_trace_kwargs)
    res = run_bass_kernel_spmd(nc, in_maps, core_ids=list(range(NCORES)), **kwargs)

    outf = np.empty((B, N, DIM), dtype=np.float32)
    for c in range(NCORES):
        bi, u = c // 2, c % 2
        outf[bi, u * NQ:(u + 1) * NQ, :] = res.results[c]["out"]
    if _trace:
        return outf, res
    return outf
